# revision 24
# baseline (speedup 1.0000x reference)
"""Trainium2 Bass kernel for multi-head attention (B=8, N=1024, DM=512, H=8, D=64).

Sharding: data-parallel over batch — core i handles batch element i, weights
replicated, no collectives.

Per-core pipeline (all matmul operands float32r — tf32-like, 1 cyc/row):
  - host feeds current^T / hidden^T [512, 1024]
  - qT, kT = W^T @ x^T   (weights stationary)      [512 rows, 1024 tokens]
  - v natural [1024, 512] (hiddenT chunks stationary), stored ones-augmented
    as [128, 8, 65] per token-chunk (col 64 = 1.0 -> softmax sums for free)
  - per (q-chunk, head-pair): dotsT [keys, queries], two heads packed
    concurrently in the PE array (K=64 row tiling, banks 0/1 of a 2-bank
    PSUM tile); exp on ScalarE (no max subtraction -- logits are O(+-50),
    fp32 exp cannot overflow) -> SBUF f32r
  - PV: out_h^T[65, 512] = v_aug^T @ expT, row 64 = softmax denominators
  - normalize: gather sums [1,512]->[8,64] (DMA), DVE reciprocal, scatter
    back to a row, GpSimd partition-broadcast [64,512], DVE multiply
  - out-proj: out[t-chunk, :] = sum_ic oT[ic, t-chunk]^T @ Wo[ic, :]
"""
import sys

sys.path.insert(0, "/opt/trn_rl_repo")

import numpy as np

import concourse.bass as bass  # noqa: F401  (import keeps bass registered)
import concourse.mybir as mybir
import concourse.tile as tile
from concourse import bacc
from concourse.bass_utils import run_bass_kernel_spmd

F32 = mybir.dt.float32
F32R = mybir.dt.float32r

B, N, DM = 8, 1024, 512
H, D = 8, 64
NCORES = 8

_nc_cache = {}


def build_nc(loop_n=None):
    key = ("nc", loop_n)
    if key in _nc_cache:
        return _nc_cache[key]
    nc = bacc.Bacc("TRN2", target_bir_lowering=False, debug=False)

    curT_d = nc.dram_tensor("currentT", [DM, N], F32R, kind="ExternalInput").ap()
    hidT_d = nc.dram_tensor("hiddenT", [DM, N], F32R, kind="ExternalInput").ap()
    wq_d = nc.dram_tensor("Wq", [DM, H * D], F32R, kind="ExternalInput").ap()
    wkv_d = nc.dram_tensor("Wkv", [DM, 2 * H * D], F32R, kind="ExternalInput").ap()
    wo_d = nc.dram_tensor("Wo", [H * D, H * D], F32R, kind="ExternalInput").ap()
    ones_d = nc.dram_tensor("ones", [128, H], F32R, kind="ExternalInput").ap()
    out_d = nc.dram_tensor("out", [N, H * D], F32, kind="ExternalOutput").ap()

    with tile.TileContext(nc) as tc:
        if loop_n is None:
            build_body(nc, tc, curT_d, hidT_d, wq_d, wkv_d, wo_d, ones_d, out_d)
        else:
            with tc.For_i(0, loop_n, 1):
                build_body(nc, tc, curT_d, hidT_d, wq_d, wkv_d, wo_d, ones_d,
                           out_d)
    nc.compile()
    _nc_cache[key] = nc
    return nc


CONFIG = {"dots_banks": 2}


def build_body(nc, tc, curT_d, hidT_d, wq_d, wkv_d, wo_d, ones_d, out_d):
    import contextlib

    dots_banks = CONFIG["dots_banks"]
    kpt = dots_banks // 2          # k-chunks per dots tile
    ntiles = 8 // kpt              # dots tiles per pair-job
    dots_bufs = 4 // dots_banks if dots_banks == 4 else 2

    ctx = contextlib.ExitStack()
    with ctx:
        # ---------- pools ----------
        wpool = ctx.enter_context(tc.tile_pool(name="weights", bufs=1))
        actpool = ctx.enter_context(tc.tile_pool(name="acts", bufs=1))
        qkpool = ctx.enter_context(tc.tile_pool(name="qk", bufs=1))
        vpool = ctx.enter_context(tc.tile_pool(name="vaug", bufs=1))
        opool = ctx.enter_context(tc.tile_pool(name="ot", bufs=1))
        epool = ctx.enter_context(tc.tile_pool(name="expT", bufs=24 // dots_banks))
        pvsb = ctx.enter_context(tc.tile_pool(name="pvsb", bufs=3))
        stgpool = ctx.enter_context(tc.tile_pool(name="stg", bufs=2))
        smallp = ctx.enter_context(tc.tile_pool(name="small", bufs=3))
        bcpool = ctx.enter_context(tc.tile_pool(name="bcast", bufs=3))
        outsb = ctx.enter_context(tc.tile_pool(name="outsb", bufs=2))
        dpsum = ctx.enter_context(
            tc.tile_pool(name="dpsum", bufs=dots_bufs, space="PSUM"))
        pvps = ctx.enter_context(tc.tile_pool(name="pvps", bufs=2, space="PSUM"))
        ppsum_cm = tc.tile_pool(name="ppsum", bufs=1, space="PSUM")
        ppsum = ppsum_cm.__enter__()
        state = {"opps": None, "ppsum_open": True}

        # ---------- input loads (spread across DMA queues) ----------
        wq = [wpool.tile([128, 512], F32R, tag=f"wq{k}", name=f"wq{k}") for k in range(4)]
        wkv = [wpool.tile([128, 1024], F32R, tag=f"wkv{k}", name=f"wkv{k}") for k in range(4)]
        wo = [wpool.tile([128, 512], F32R, tag=f"wo{k}", name=f"wo{k}") for k in range(4)]
        curT = [actpool.tile([128, 1024], F32R, tag=f"cur{k}", name=f"cur{k}") for k in range(4)]
        hidT = [actpool.tile([128, 1024], F32R, tag=f"hid{k}", name=f"hid{k}") for k in range(4)]

        hw = [nc.sync, nc.scalar]
        for k in range(4):
            hw[k % 2].dma_start(wq[k][:], wq_d[k * 128:(k + 1) * 128, :])
            hw[(k + 1) % 2].dma_start(curT[k][:], curT_d[k * 128:(k + 1) * 128, :])
        for k in range(4):
            hw[k % 2].dma_start(wkv[k][:], wkv_d[k * 128:(k + 1) * 128, :])
            hw[(k + 1) % 2].dma_start(hidT[k][:], hidT_d[k * 128:(k + 1) * 128, :])
        for k in range(4):
            hw[k % 2].dma_start(wo[k][:], wo_d[k * 128:(k + 1) * 128, :])

        # ---------- persistent tensors ----------
        qT = [qkpool.tile([128, 1024], F32R, tag=f"qT{m}", name=f"qT{m}") for m in range(4)]
        kT = [qkpool.tile([128, 1024], F32R, tag=f"kT{m}", name=f"kT{m}") for m in range(4)]
        vaug = [vpool.tile([128, H, D + 1], F32R, tag=f"va{t}", name=f"va{t}") for t in range(8)]
        oT = [opool.tile([128, 1024], F32R, tag=f"oT{i}", name=f"oT{i}") for i in range(4)]

        # ---------- projection units (emitted eagerly or as fillers) ----------
        def emit_qproj(m):
            ps = ppsum.tile([128, 2, 512], F32, tag="proj", name="psq")
            for k in range(4):
                for n2 in range(2):
                    nc.tensor.matmul(
                        ps[:, n2, :],
                        wq[k][:, m * 128:(m + 1) * 128],
                        curT[k][:, n2 * 512:(n2 + 1) * 512],
                        start=(k == 0), stop=(k == 3))
            nc.vector.tensor_copy(qT[m][:].rearrange("p (a b) -> p a b", a=2), ps[:])

        def emit_kproj(m):
            ps = ppsum.tile([128, 2, 512], F32, tag="proj", name="psk")
            for k in range(4):
                for n2 in range(2):
                    nc.tensor.matmul(
                        ps[:, n2, :],
                        wkv[k][:, m * 128:(m + 1) * 128],
                        hidT[k][:, n2 * 512:(n2 + 1) * 512],
                        start=(k == 0), stop=(k == 3))
            nc.vector.tensor_copy(kT[m][:].rearrange("p (a b) -> p a b", a=2), ps[:])

        def emit_vproj(tp):
            ps = ppsum.tile([128, 2, 512], F32, tag="proj", name="psv")
            for t2 in range(2):
                tc_i = tp * 2 + t2
                for k in range(4):
                    nc.tensor.matmul(
                        ps[:, t2, :],
                        hidT[k][:, tc_i * 128:(tc_i + 1) * 128],
                        wkv[k][:, 512:1024],
                        start=(k == 0), stop=(k == 3))
            for t2 in range(2):
                tc_i = tp * 2 + t2
                nc.vector.tensor_copy(
                    vaug[tc_i][:, :, 0:D],
                    ps[:, t2, :].rearrange("p (h d) -> p h d", h=H))
                nc.sync.dma_start(vaug[tc_i][:, :, D:D + 1], ones_d[:])

        # ---------- attention helpers ----------
        def emit_pv(js, kc):
            if kc == 0:
                js["pve"] = pvps.tile([D + 1, 512], F32, tag="pv", name="pve")
                js["pvo"] = pvps.tile([D + 1, 512], F32, tag="pv", name="pvo")
            et = js["etiles"][kc // kpt]
            j = kc % kpt
            hp = js["hp"]
            nc.tensor.matmul(js["pve"][:], vaug[kc][:, 2 * hp, :],
                             et[:, 2 * j, :], start=(kc == 0), stop=(kc == 7))
            nc.tensor.matmul(js["pvo"][:], vaug[kc][:, 2 * hp + 1, :],
                             et[:, 2 * j + 1, :], start=(kc == 0), stop=(kc == 7))

        def emit_pv_tile(js, kt):
            for j in range(kpt):
                emit_pv(js, kpt * kt + j)

        def emit_norm(js):
            qc, hp = js["qc"], js["hp"]
            psb_e = pvsb.tile([D + 1, 512], F32, tag="pvsb", name="psbe")
            nc.vector.tensor_copy(psb_e[:], js["pve"][:])
            psb_o = pvsb.tile([D + 1, 512], F32, tag="pvsb", name="psbo")
            nc.vector.tensor_copy(psb_o[:], js["pvo"][:])
            gath = smallp.tile([16, 64], F32, tag="gath", name="gath")
            nc.sync.dma_start(gath[0:8, :], psb_e[D:D + 1, :])
            nc.sync.dma_start(gath[8:16, :], psb_o[D:D + 1, :])
            rec = smallp.tile([16, 64], F32, tag="rec", name="rec")
            nc.vector.reciprocal(rec[:], gath[:])
            for h2, psb in ((0, psb_e), (1, psb_o)):
                rrow = smallp.tile([1, 512], F32, tag="rrow", name="rrow")
                nc.sync.dma_start(rrow[:], rec[8 * h2:8 * h2 + 8, :])
                bc = bcpool.tile([64, 512], F32, tag="bc", name="bc")
                nc.gpsimd.partition_broadcast(bc[:], rrow[:])
                if h2 == 0:
                    nc.vector.tensor_mul(
                        oT[hp][0:64, qc * 512:(qc + 1) * 512],
                        psb[0:D, :], bc[:])
                else:
                    # DVE lanes cannot cross partitions: compute in 0:64,
                    # then DMA into partitions 64:128 of the oT tile.
                    stg = stgpool.tile([64, 512], F32R, tag="stg", name="stg")
                    nc.vector.tensor_mul(stg[:], psb[0:D, :], bc[:])
                    nc.sync.dma_start(
                        oT[hp][64:128, qc * 512:(qc + 1) * 512], stg[:])

        def emit_outproj(qc):
            if state["opps"] is None:
                state["opps"] = ctx.enter_context(
                    tc.tile_pool(name="opps", bufs=2, space="PSUM"))
            for t2 in range(4):
                tc_i = qc * 4 + t2
                ops = state["opps"].tile([128, 512], F32, tag="op", name="ops")
                for ic in range(4):
                    nc.tensor.matmul(
                        ops[:],
                        oT[ic][:, tc_i * 128:(tc_i + 1) * 128],
                        wo[ic][:],
                        start=(ic == 0), stop=(ic == 3))
                osb = outsb.tile([128, 512], F32, tag="osb", name="osb")
                nc.vector.tensor_copy(osb[:], ops[:])
                nc.sync.dma_start(out_d[tc_i * 128:(tc_i + 1) * 128, :], osb[:])

        # ---------- schedule ----------
        # eager: q/k projections for head-pair 0; the rest become fillers
        emit_qproj(0)
        emit_kproj(0)
        fillers = [lambda tp=tp: emit_vproj(tp) for tp in range(4)]
        for m in range(1, 4):
            fillers.append(lambda m=m: emit_qproj(m))
            fillers.append(lambda m=m: emit_kproj(m))
        fillers = fillers[::-1]  # pop() from the front

        # job order: qk[m] is ready exactly when head-pair m first runs
        jobs = [(0, 0), (1, 0), (0, 1), (0, 2), (0, 3), (1, 1), (1, 2), (1, 3)]

        pending = None
        outproj_pending = None
        for ji, (qc, hp) in enumerate(jobs):
            etiles = [epool.tile([128, dots_banks, 512], F32R, tag="exp",
                                 name="exp") for _ in range(ntiles)]
            cur = {"qc": qc, "hp": hp, "etiles": etiles, "pve": None,
                   "pvo": None}
            if not fillers and state["ppsum_open"]:
                state["ppsum_open"] = False
                ppsum_cm.__exit__(None, None, None)
            for kt in range(ntiles):
                dp = dpsum.tile([128, dots_banks, 512], F32, tag="dps",
                                name="dps")
                for j in range(kpt):
                    kc = kpt * kt + j
                    # two heads concurrently: row groups 0:64 and 64:128
                    nc.tensor.matmul(
                        dp[:, 2 * j, :],
                        kT[hp][0:64, kc * 128:(kc + 1) * 128],
                        qT[hp][0:64, qc * 512:(qc + 1) * 512],
                        start=True, stop=True)
                    nc.tensor.matmul(
                        dp[:, 2 * j + 1, :],
                        kT[hp][64:128, kc * 128:(kc + 1) * 128],
                        qT[hp][64:128, qc * 512:(qc + 1) * 512],
                        start=True, stop=True)
                nc.scalar.activation(etiles[kt][:], dp[:],
                                     mybir.ActivationFunctionType.Exp)
                # PV of the previous job overlaps this tile's ACT exp
                if pending is not None:
                    emit_pv_tile(pending, kt)
                if fillers:
                    fillers.pop()()
            if pending is not None:
                emit_norm(pending)
                if pending["qc"] == 0 and pending["hp"] == 3:
                    outproj_pending = 0
            pending = cur
            if outproj_pending is not None:
                emit_outproj(outproj_pending)
                outproj_pending = None
        # drain last job
        for kt in range(ntiles):
            emit_pv_tile(pending, kt)
        emit_norm(pending)
        emit_outproj(1)


def make_in_maps(inputs):
    current = np.asarray(inputs["current"], dtype=np.float32)
    hidden = np.asarray(inputs["hidden"], dtype=np.float32)
    Wq = np.ascontiguousarray(np.asarray(inputs["Wq"], dtype=np.float32))
    Wkv = np.ascontiguousarray(np.asarray(inputs["Wkv"], dtype=np.float32))
    Wo = np.ascontiguousarray(np.asarray(inputs["Wo"], dtype=np.float32))
    ones = np.ones((128, H), dtype=np.float32)

    in_maps = []
    for i in range(NCORES):
        in_maps.append({
            "currentT": np.ascontiguousarray(current[i].T),
            "hiddenT": np.ascontiguousarray(hidden[i].T),
            "Wq": Wq, "Wkv": Wkv, "Wo": Wo, "ones": ones,
        })
    return in_maps


def kernel(current, hidden, Wq, Wkv, Wo):
    in_maps = make_in_maps(
        {"current": current, "hidden": hidden, "Wq": Wq, "Wkv": Wkv, "Wo": Wo})
    nc = build_nc()
    res = run_bass_kernel_spmd(nc, in_maps, core_ids=list(range(NCORES)))
    out = np.stack([res.results[i]["out"] for i in range(NCORES)], axis=0)
    return out
